# revision 24
# baseline (speedup 1.0000x reference)
"""Chamfer loss TRN2 kernel.

preds/gts: [8, 4096, 3] fp32. Output: [8] fp32 loss per batch sample.

Strategy: data-parallel, one batch sample per NeuronCore (8 cores).
Per core, the 4096x4096 squared-distance matrix P is computed tile-by-tile
on the TensorEngine via an augmented matmul:
    P[n,m] = ||g_n||^2 + ||p_m||^2 - 2 g_n.p_m = sum_k ga[k,n] * pa[k,m]
To run the PE at bf16 rate (4x fp32) without losing fp32 accuracy, every
fp32 operand is split into three bf16 levels covering the full mantissa,
and the K=24 contraction rows carry all hi/lo cross products of magnitude
>= 2^-27; products of bf16s are exact in the fp32 PSUM accumulation.

Per 128-row block: ScalarE extracts the PSUM tiles to SBUF fp16 (the only
fast PSUM reader); VectorE then does all min work — a full-width
tensor_tensor min into the running column-min accumulator, and a TT-min
halving chain + short reduce for the block's row mins (TT fp16 runs in
2x_1P mode; plain reduce is 1x, hence the chain).  Column mins are
finalized with PE transposes + reduce; final sums in fp32; the partition
sum is a matmul against ones.  Engines measure ~97% (DVE), ~88% (PE),
~73% (ScalarE) busy — DVE is the floor since GpSimd tensor ops, DVE
tensor_tensor_reduce, and DMA accum all fail this container's walrus.
"""

import os
import sys

sys.path.insert(0, "/opt/trn_rl_repo")

# the device path needs jax's axon backend; a cpu pin (common in bench
# templates for the *reference* side) would break device dispatch here
if os.environ.get("JAX_PLATFORMS", "").strip().lower() == "cpu":
    os.environ.pop("JAX_PLATFORMS")

import numpy as np

B = 8
N = 4096  # points per cloud
PT = 128  # partition tile (gts points per row-block)
FT = 512  # matmul free-dim tile (preds per matmul)
GRP = 4  # matmul tiles extracted per copy (PSUM banks per group)
K = 24  # contraction rows (3-level bf16 split + norms + ones)
NT = N // PT  # 32 row-blocks
NJ = N // FT  # 8 col-blocks
NH = NJ // GRP  # 2 extraction groups per row-block
GW = GRP * FT  # 2048, group width

_CACHE = {}


def _split_multiwait(nc):
    """This container's walrus rejects instructions carrying more than one
    sync wait.  For every instruction with N>1 waits, hoist N-1 of them onto
    freshly created same-engine NOPs placed immediately before it."""
    from concourse import mybir

    for bb in nc.main_func.blocks:
        il = list(bb.instructions)
        new = []
        changed = False
        for inst in il:
            si = inst.sync_info
            if si is not None and si.on_wait is not None and len(si.on_wait) > 1:
                waits = list(si.on_wait)
                eng = nc.engines.get(inst.engine)
                if eng is None:
                    new.append(inst)
                    continue
                for w in waits[:-1]:
                    nop = eng.nop(nofuse=True)
                    cur = nc.cur_bb.bb
                    cil = list(cur.instructions)
                    assert cil[-1].name == nop.ins.name
                    cur.instructions = cil[:-1]
                    nop.ins.sync_info = mybir.SyncInfo(on_wait=[w], on_update=[])
                    new.append(nop.ins)
                si.on_wait = [waits[-1]]
                changed = True
            new.append(inst)
        if changed:
            bb.instructions = new


def _patch_tile_drain():
    """Tile's exit drain accumulates one wait per live semaphore; split it,
    then run the global multi-wait splitter over the whole program."""
    import concourse.tile as tile
    from concourse import mybir
    from concourse.vector_clock import ScopedClock

    if getattr(tile.TileContext, "_drain_patched", False):
        return

    def _drain_and_barrier(self, tick_clock, wait_clock):
        nc = self.nc
        drain_inst = nc.sync.drain()
        wait_clock.add_sem_waits(
            drain_inst.ins, ScopedClock({None: tick_clock.global_clock})
        )
        si = drain_inst.ins.sync_info
        if si is not None and si.on_wait is not None and len(si.on_wait) > 1:
            waits = list(si.on_wait)
            si.on_wait = waits[:1]
            for w in waits[1:]:
                extra = nc.sync.drain()
                esi = extra.ins.sync_info
                if esi is None:
                    extra.ins.sync_info = mybir.SyncInfo(on_wait=[w], on_update=[])
                else:
                    esi.on_wait = [w]
        nc.all_engine_barrier()
        popped = nc._tile_sem_poison_stack.pop()
        assert popped is self._sem_poison
        nc.clear_and_free_semaphores(list(self.sems.allocated().values()))
        nc.all_engine_barrier()
        _split_multiwait(nc)

    tile.TileContext._drain_and_barrier = _drain_and_barrier
    tile.TileContext._drain_patched = True


def _build():
    import concourse.bass as bass
    import concourse.tile as tile
    from concourse import mybir
    from concourse.masks import make_identity

    _patch_tile_drain()

    f32 = mybir.dt.float32
    f16 = mybir.dt.float16
    bf16 = mybir.dt.bfloat16
    AX = mybir.AxisListType
    OP = mybir.AluOpType

    nc = bass.Bass()
    ga = nc.declare_dram_parameter("ga", [K, N], bf16, isOutput=False)
    pa = nc.declare_dram_parameter("pa", [K, N], bf16, isOutput=False)
    loss = nc.declare_dram_parameter("loss", [1, 1], f32, isOutput=True)

    with tile.TileContext(nc) as tc:
        with (
            tc.tile_pool(name="consts", bufs=1) as consts,
            tc.tile_pool(name="staged", bufs=3) as staged_pool,
            tc.tile_pool(name="halv", bufs=2) as halv_pool,
            tc.tile_pool(name="accs", bufs=1) as accs,
            tc.tile_pool(name="sums", bufs=2) as sums,
        ):
            ga_s = consts.tile([K, N], bf16)
            pa_s = consts.tile([K, N], bf16)
            nc.sync.dma_start(out=ga_s[:], in_=ga[:])
            nc.sync.dma_start(out=pa_s[:], in_=pa[:])
            ident = consts.tile([PT, PT], f16)
            make_identity(nc, ident[:])

            # running column-min over row-blocks, fp16 [128, 4096]
            colacc = accs.tile([PT, N], f16)
            # per-row-block row mins
            rowstage = accs.tile([PT, NT], f32)

            TB = 4  # row-blocks whose row-min chains run as one batched op set

            def emit_chain(st4, nblk, t0):
                """Row mins for nblk staged row-blocks: TT-min halving chain
                on 3D APs (2x_1P mode; batching amortizes per-op DRAIN)."""
                hA = halv_pool.tile([PT, TB, GW], f16, tag="hA")
                nc.vector.tensor_tensor(
                    out=hA[:, :nblk, :],
                    in0=st4[:, :nblk, :GW],
                    in1=st4[:, :nblk, GW:],
                    op=OP.min,
                )
                hB = halv_pool.tile([PT, TB, GW // 2], f16, tag="hB")
                nc.vector.tensor_tensor(
                    out=hB[:, :nblk, :],
                    in0=hA[:, :nblk, : GW // 2],
                    in1=hA[:, :nblk, GW // 2 :],
                    op=OP.min,
                )
                hC = halv_pool.tile([PT, TB, GW // 4], f16, tag="hC")
                nc.vector.tensor_tensor(
                    out=hC[:, :nblk, :],
                    in0=hB[:, :nblk, : GW // 4],
                    in1=hB[:, :nblk, GW // 4 :],
                    op=OP.min,
                )
                hD = halv_pool.tile([PT, TB, GW // 8], f16, tag="hD")
                nc.vector.tensor_tensor(
                    out=hD[:, :nblk, :],
                    in0=hC[:, :nblk, : GW // 8],
                    in1=hC[:, :nblk, GW // 8 :],
                    op=OP.min,
                )
                nc.vector.tensor_reduce(
                    out=rowstage[:, t0 : t0 + nblk],
                    in_=hD[:, :nblk, :],
                    axis=AX.X,
                    op=OP.min,
                )

            # chain flush points: uniform TB-wide batches (staggering the last
            # blocks into singles was measured slower — the scheduler already
            # overlaps the final chain with the column-min finalization)
            flush_at = {TB * i + TB - 1 for i in range(NT // TB)}
            with tc.tile_pool(name="psum_mm", bufs=2, space="PSUM") as psum_mm:
                batch_start = 0
                st4 = None
                for t in range(NT):
                    if st4 is None:
                        st4 = staged_pool.tile([PT, TB, N], f16, tag="st")
                        batch_start = t
                    tt = t - batch_start
                    lhsT = ga_s[:, t * PT : (t + 1) * PT]
                    # t=0 uses finer sub-groups (2 matmuls + 1024-wide copy)
                    # so the first extraction lands ~2us earlier and VectorE
                    # starts sooner; one-time +0.6us of ScalarE
                    ngrp = 4 if t == 0 else NH
                    gw = N // ngrp
                    for h in range(ngrp):
                        ps = psum_mm.tile([PT, GW], f32, tag="mm")
                        for g in range(gw // FT):
                            j = (h * gw) // FT + g
                            nc.tensor.matmul(
                                out=ps[:, g * FT : (g + 1) * FT],
                                lhsT=lhsT,
                                rhs=pa_s[:, j * FT : (j + 1) * FT],
                                start=True,
                                stop=True,
                            )
                        # PSUM -> SBUF extraction + fp16 downcast, ScalarE
                        nc.scalar.copy(
                            out=st4[:, tt, h * gw : (h + 1) * gw], in_=ps[:, :gw]
                        )
                    # column-min accumulate, one full-width TT (2x mode);
                    # per-t interleave keeps DVE busy during extraction (a
                    # batched pair-tree at the flush measured slower — it
                    # bursts DVE work and stalls the 2-buffer pipeline)
                    if t == 0:
                        nc.vector.tensor_copy(
                            out=colacc[:, : N // 2], in_=st4[:, 0, : N // 2]
                        )
                        nc.vector.tensor_copy(
                            out=colacc[:, N // 2 :], in_=st4[:, 0, N // 2 :]
                        )
                    else:
                        nc.vector.tensor_tensor(
                            out=colacc[:], in0=colacc[:], in1=st4[:, tt, :], op=OP.min
                        )
                    if t in flush_at:
                        emit_chain(st4, tt + 1, batch_start)
                        st4 = None

            with (
                tc.tile_pool(name="psum_tail", bufs=2, space="PSUM") as psum_tail,
                tc.tile_pool(name="psum_fin", bufs=1, space="PSUM") as psum_fin,
            ):
                # column mins: transpose 128-wide blocks (16 per PSUM tile),
                # reduce each transposed block over its n-residuals
                colmin = accs.tile([PT, NT], f32)
                for k16 in range(NT // 16):
                    pst = psum_tail.tile([PT, 16, PT], f16, tag="tr")
                    for i in range(16):
                        k = k16 * 16 + i
                        nc.tensor.transpose(
                            out=pst[:, i, :],
                            in_=colacc[:, k * PT : (k + 1) * PT],
                            identity=ident[:],
                        )
                    nc.vector.tensor_reduce(
                        out=colmin[:, k16 * 16 : (k16 + 1) * 16],
                        in_=pst[:],
                        axis=AX.X,
                        op=OP.min,
                    )

                r1 = sums.tile([PT, 1], f32)
                nc.vector.tensor_reduce(out=r1[:], in_=colmin[:], axis=AX.X, op=OP.add)
                r2 = sums.tile([PT, 1], f32)
                nc.vector.tensor_reduce(out=r2[:], in_=rowstage[:], axis=AX.X, op=OP.add)
                r = sums.tile([PT, 1], f32)
                nc.vector.tensor_add(out=r[:], in0=r1[:], in1=r2[:])

                ones = consts.tile([PT, 1], f32)
                nc.vector.memset(ones[:], 1.0)
                pscal = psum_fin.tile([1, 1], f32)
                nc.tensor.matmul(
                    out=pscal[:], lhsT=r[:], rhs=ones[:], start=True, stop=True
                )
                loss_s = sums.tile([1, 1], f32)
                nc.scalar.copy(out=loss_s[:], in_=pscal[:])
                nc.sync.dma_start(out=loss[:], in_=loss_s[:])

    return nc


def _bf16_split3(x):
    """Split fp32 array into three bf16 levels covering the full mantissa."""
    import ml_dtypes

    bf = ml_dtypes.bfloat16
    a = x.astype(bf)
    r = x - a.astype(np.float32)
    b = r.astype(bf)
    c = (r - b.astype(np.float32)).astype(bf)
    return a, b, c


def _prep(preds, gts):
    """Host-side augmentation: per sample, [K, N] bf16 hi/lo operands."""
    import ml_dtypes

    bf = ml_dtypes.bfloat16
    in_maps = []
    for b in range(B):
        g = np.asarray(gts[b], dtype=np.float32)
        p = np.asarray(preds[b], dtype=np.float32)
        q = -2.0 * p
        g1, g2, g3 = _bf16_split3(g.T)  # [3, N] each
        q1, q2, q3 = _bf16_split3(q.T)
        rx = (g * g).sum(axis=1, dtype=np.float32)
        ry = (p * p).sum(axis=1, dtype=np.float32)
        rx1, rx2, rx3 = _bf16_split3(rx)
        ry1, ry2, ry3 = _bf16_split3(ry)
        one = np.ones((1, N), dtype=bf)

        # pair (lhs row, rhs row) so the contraction carries every hi/lo
        # cross term of magnitude >= 2^-27: g.q needs g1q1, g1q2, g2q1,
        # g1q3, g2q2, g3q1.
        ga = np.empty((K, N), dtype=bf)
        pa = np.empty((K, N), dtype=bf)
        for i, (gr, qr) in enumerate(
            [(g1, q1), (g1, q2), (g2, q1), (g1, q3), (g2, q2), (g3, q1)]
        ):
            ga[3 * i : 3 * i + 3] = gr
            pa[3 * i : 3 * i + 3] = qr
        ga[18], ga[19], ga[20] = rx1, rx2, rx3
        pa[18:21] = one
        ga[21:24] = one
        pa[21], pa[22], pa[23] = ry1, ry2, ry3
        in_maps.append({"ga": ga, "pa": pa})
    return in_maps


def kernel(preds, gts):
    from concourse.bass_utils import run_bass_kernel_spmd

    if "nc" not in _CACHE:
        _CACHE["nc"] = _build()
    nc = _CACHE["nc"]
    in_maps = _prep(preds, gts)
    res = run_bass_kernel_spmd(nc, in_maps, core_ids=list(range(B)))
    out = np.array(
        [res.results[b]["loss"][0, 0] for b in range(B)], dtype=np.float32
    )
    return out


# revision 25
# speedup vs baseline: 1.0068x; 1.0068x over previous
"""Chamfer loss TRN2 kernel.

preds/gts: [8, 4096, 3] fp32. Output: [8] fp32 loss per batch sample.

Strategy: data-parallel, one batch sample per NeuronCore (8 cores).
Per core, the 4096x4096 squared-distance matrix P is computed tile-by-tile
on the TensorEngine via an augmented matmul:
    P[n,m] = ||g_n||^2 + ||p_m||^2 - 2 g_n.p_m = sum_k ga[k,n] * pa[k,m]
To run the PE at bf16 rate (4x fp32) without losing fp32 accuracy, every
fp32 operand is split into three bf16 levels covering the full mantissa,
and the K=24 contraction rows carry all hi/lo cross products of magnitude
>= 2^-27; products of bf16s are exact in the fp32 PSUM accumulation.

Per 128-row block: ScalarE extracts the PSUM tiles to SBUF fp16 (the only
fast PSUM reader); VectorE then does all min work — a full-width
tensor_tensor min into the running column-min accumulator, and a TT-min
halving chain + short reduce for the block's row mins (TT fp16 runs in
2x_1P mode; plain reduce is 1x, hence the chain).  Column mins are
finalized with PE transposes + reduce; final sums in fp32; the partition
sum is a matmul against ones.  Engines measure ~97% (DVE), ~88% (PE),
~73% (ScalarE) busy — DVE is the floor since GpSimd tensor ops, DVE
tensor_tensor_reduce, and DMA accum all fail this container's walrus.
"""

import os
import sys

sys.path.insert(0, "/opt/trn_rl_repo")

# the device path needs jax's axon backend; a cpu pin (common in bench
# templates for the *reference* side) would break device dispatch here
if os.environ.get("JAX_PLATFORMS", "").strip().lower() == "cpu":
    os.environ.pop("JAX_PLATFORMS")

import numpy as np

B = 8
N = 4096  # points per cloud
PT = 128  # partition tile (gts points per row-block)
FT = 512  # matmul free-dim tile (preds per matmul)
GRP = 4  # matmul tiles extracted per copy (PSUM banks per group)
K = 24  # contraction rows (3-level bf16 split + norms + ones)
NT = N // PT  # 32 row-blocks
NJ = N // FT  # 8 col-blocks
NH = NJ // GRP  # 2 extraction groups per row-block
GW = GRP * FT  # 2048, group width

_CACHE = {}


def _split_multiwait(nc):
    """This container's walrus rejects instructions carrying more than one
    sync wait.  For every instruction with N>1 waits, hoist N-1 of them onto
    freshly created same-engine NOPs placed immediately before it."""
    from concourse import mybir

    for bb in nc.main_func.blocks:
        il = list(bb.instructions)
        new = []
        changed = False
        for inst in il:
            si = inst.sync_info
            if si is not None and si.on_wait is not None and len(si.on_wait) > 1:
                waits = list(si.on_wait)
                eng = nc.engines.get(inst.engine)
                if eng is None:
                    new.append(inst)
                    continue
                for w in waits[:-1]:
                    nop = eng.nop(nofuse=True)
                    cur = nc.cur_bb.bb
                    cil = list(cur.instructions)
                    assert cil[-1].name == nop.ins.name
                    cur.instructions = cil[:-1]
                    nop.ins.sync_info = mybir.SyncInfo(on_wait=[w], on_update=[])
                    new.append(nop.ins)
                si.on_wait = [waits[-1]]
                changed = True
            new.append(inst)
        if changed:
            bb.instructions = new


def _patch_tile_drain():
    """Tile's exit drain accumulates one wait per live semaphore; split it,
    then run the global multi-wait splitter over the whole program."""
    import concourse.tile as tile
    from concourse import mybir
    from concourse.vector_clock import ScopedClock

    if getattr(tile.TileContext, "_drain_patched", False):
        return

    def _drain_and_barrier(self, tick_clock, wait_clock):
        nc = self.nc
        drain_inst = nc.sync.drain()
        wait_clock.add_sem_waits(
            drain_inst.ins, ScopedClock({None: tick_clock.global_clock})
        )
        si = drain_inst.ins.sync_info
        if si is not None and si.on_wait is not None and len(si.on_wait) > 1:
            waits = list(si.on_wait)
            si.on_wait = waits[:1]
            for w in waits[1:]:
                extra = nc.sync.drain()
                esi = extra.ins.sync_info
                if esi is None:
                    extra.ins.sync_info = mybir.SyncInfo(on_wait=[w], on_update=[])
                else:
                    esi.on_wait = [w]
        nc.all_engine_barrier()
        popped = nc._tile_sem_poison_stack.pop()
        assert popped is self._sem_poison
        nc.clear_and_free_semaphores(list(self.sems.allocated().values()))
        nc.all_engine_barrier()
        _split_multiwait(nc)

    tile.TileContext._drain_and_barrier = _drain_and_barrier
    tile.TileContext._drain_patched = True


def _build():
    import concourse.bass as bass
    import concourse.tile as tile
    from concourse import mybir
    from concourse.masks import make_identity

    _patch_tile_drain()

    f32 = mybir.dt.float32
    f16 = mybir.dt.float16
    bf16 = mybir.dt.bfloat16
    AX = mybir.AxisListType
    OP = mybir.AluOpType

    nc = bass.Bass()
    ga = nc.declare_dram_parameter("ga", [K, N], bf16, isOutput=False)
    pa = nc.declare_dram_parameter("pa", [K, N], bf16, isOutput=False)
    loss = nc.declare_dram_parameter("loss", [1, 1], f32, isOutput=True)

    with tile.TileContext(nc) as tc:
        with (
            tc.tile_pool(name="consts", bufs=1) as consts,
            tc.tile_pool(name="staged", bufs=3) as staged_pool,
            tc.tile_pool(name="halv", bufs=2) as halv_pool,
            tc.tile_pool(name="accs", bufs=1) as accs,
            tc.tile_pool(name="sums", bufs=2) as sums,
        ):
            ga_s = consts.tile([K, N], bf16)
            pa_s = consts.tile([K, N], bf16)
            nc.sync.dma_start(out=ga_s[:], in_=ga[:])
            nc.sync.dma_start(out=pa_s[:], in_=pa[:])
            ident = consts.tile([PT, PT], f16)
            make_identity(nc, ident[:])

            # running column-min over row-blocks, fp16 [128, 4096]
            colacc = accs.tile([PT, N], f16)
            # per-row-block row mins
            rowstage = accs.tile([PT, NT], f32)

            TB = 4  # row-blocks whose row-min chains run as one batched op set

            def emit_chain(st4, nblk, t0):
                """Row mins for nblk staged row-blocks: TT-min halving chain
                on 3D APs (2x_1P mode; batching amortizes per-op DRAIN)."""
                hA = halv_pool.tile([PT, TB, GW], f16, tag="hA")
                nc.vector.tensor_tensor(
                    out=hA[:, :nblk, :],
                    in0=st4[:, :nblk, :GW],
                    in1=st4[:, :nblk, GW:],
                    op=OP.min,
                )
                hB = halv_pool.tile([PT, TB, GW // 2], f16, tag="hB")
                nc.vector.tensor_tensor(
                    out=hB[:, :nblk, :],
                    in0=hA[:, :nblk, : GW // 2],
                    in1=hA[:, :nblk, GW // 2 :],
                    op=OP.min,
                )
                hC = halv_pool.tile([PT, TB, GW // 4], f16, tag="hC")
                nc.vector.tensor_tensor(
                    out=hC[:, :nblk, :],
                    in0=hB[:, :nblk, : GW // 4],
                    in1=hB[:, :nblk, GW // 4 :],
                    op=OP.min,
                )
                hD = halv_pool.tile([PT, TB, GW // 8], f16, tag="hD")
                nc.vector.tensor_tensor(
                    out=hD[:, :nblk, :],
                    in0=hC[:, :nblk, : GW // 8],
                    in1=hC[:, :nblk, GW // 8 :],
                    op=OP.min,
                )
                nc.vector.tensor_reduce(
                    out=rowstage[:, t0 : t0 + nblk],
                    in_=hD[:, :nblk, :],
                    axis=AX.X,
                    op=OP.min,
                )

            # chain flush points: uniform TB-wide batches (staggering the last
            # blocks into singles was measured slower — the scheduler already
            # overlaps the final chain with the column-min finalization)
            flush_at = {TB * i + TB - 1 for i in range(NT // TB)}
            with tc.tile_pool(name="psum_mm", bufs=2, space="PSUM") as psum_mm:
                batch_start = 0
                st4 = None
                for t in range(NT):
                    if st4 is None:
                        st4 = staged_pool.tile([PT, TB, N], f16, tag="st")
                        batch_start = t
                    tt = t - batch_start
                    lhsT = ga_s[:, t * PT : (t + 1) * PT]
                    for h in range(NH):
                        ps = psum_mm.tile([PT, GW], f32, tag="mm")
                        for g in range(GRP):
                            j = h * GRP + g
                            nc.tensor.matmul(
                                out=ps[:, g * FT : (g + 1) * FT],
                                lhsT=lhsT,
                                rhs=pa_s[:, j * FT : (j + 1) * FT],
                                start=True,
                                stop=True,
                            )
                        # PSUM -> SBUF extraction + fp16 downcast, ScalarE
                        nc.scalar.copy(
                            out=st4[:, tt, h * GW : (h + 1) * GW], in_=ps[:]
                        )
                    # column-min accumulate, one full-width TT (2x mode);
                    # per-t interleave keeps DVE busy during extraction (a
                    # batched pair-tree at the flush measured slower — it
                    # bursts DVE work and stalls the 2-buffer pipeline)
                    if t == 0:
                        nc.vector.tensor_copy(out=colacc[:], in_=st4[:, 0, :])
                    else:
                        nc.vector.tensor_tensor(
                            out=colacc[:], in0=colacc[:], in1=st4[:, tt, :], op=OP.min
                        )
                    if t in flush_at:
                        emit_chain(st4, tt + 1, batch_start)
                        st4 = None

            with (
                tc.tile_pool(name="psum_tail", bufs=2, space="PSUM") as psum_tail,
                tc.tile_pool(name="psum_fin", bufs=1, space="PSUM") as psum_fin,
            ):
                # column mins: transpose 128-wide blocks (16 per PSUM tile),
                # reduce each transposed block over its n-residuals
                colmin = accs.tile([PT, NT], f32)
                for k16 in range(NT // 16):
                    pst = psum_tail.tile([PT, 16, PT], f16, tag="tr")
                    for i in range(16):
                        k = k16 * 16 + i
                        nc.tensor.transpose(
                            out=pst[:, i, :],
                            in_=colacc[:, k * PT : (k + 1) * PT],
                            identity=ident[:],
                        )
                    nc.vector.tensor_reduce(
                        out=colmin[:, k16 * 16 : (k16 + 1) * 16],
                        in_=pst[:],
                        axis=AX.X,
                        op=OP.min,
                    )

                r1 = sums.tile([PT, 1], f32)
                nc.vector.tensor_reduce(out=r1[:], in_=colmin[:], axis=AX.X, op=OP.add)
                r2 = sums.tile([PT, 1], f32)
                nc.vector.tensor_reduce(out=r2[:], in_=rowstage[:], axis=AX.X, op=OP.add)
                r = sums.tile([PT, 1], f32)
                nc.vector.tensor_add(out=r[:], in0=r1[:], in1=r2[:])

                ones = consts.tile([PT, 1], f32)
                nc.vector.memset(ones[:], 1.0)
                pscal = psum_fin.tile([1, 1], f32)
                nc.tensor.matmul(
                    out=pscal[:], lhsT=r[:], rhs=ones[:], start=True, stop=True
                )
                loss_s = sums.tile([1, 1], f32)
                nc.scalar.copy(out=loss_s[:], in_=pscal[:])
                nc.sync.dma_start(out=loss[:], in_=loss_s[:])

    return nc


def _bf16_split3(x):
    """Split fp32 array into three bf16 levels covering the full mantissa."""
    import ml_dtypes

    bf = ml_dtypes.bfloat16
    a = x.astype(bf)
    r = x - a.astype(np.float32)
    b = r.astype(bf)
    c = (r - b.astype(np.float32)).astype(bf)
    return a, b, c


def _prep(preds, gts):
    """Host-side augmentation: per sample, [K, N] bf16 hi/lo operands."""
    import ml_dtypes

    bf = ml_dtypes.bfloat16
    in_maps = []
    for b in range(B):
        g = np.asarray(gts[b], dtype=np.float32)
        p = np.asarray(preds[b], dtype=np.float32)
        q = -2.0 * p
        g1, g2, g3 = _bf16_split3(g.T)  # [3, N] each
        q1, q2, q3 = _bf16_split3(q.T)
        rx = (g * g).sum(axis=1, dtype=np.float32)
        ry = (p * p).sum(axis=1, dtype=np.float32)
        rx1, rx2, rx3 = _bf16_split3(rx)
        ry1, ry2, ry3 = _bf16_split3(ry)
        one = np.ones((1, N), dtype=bf)

        # pair (lhs row, rhs row) so the contraction carries every hi/lo
        # cross term of magnitude >= 2^-27: g.q needs g1q1, g1q2, g2q1,
        # g1q3, g2q2, g3q1.
        ga = np.empty((K, N), dtype=bf)
        pa = np.empty((K, N), dtype=bf)
        for i, (gr, qr) in enumerate(
            [(g1, q1), (g1, q2), (g2, q1), (g1, q3), (g2, q2), (g3, q1)]
        ):
            ga[3 * i : 3 * i + 3] = gr
            pa[3 * i : 3 * i + 3] = qr
        ga[18], ga[19], ga[20] = rx1, rx2, rx3
        pa[18:21] = one
        ga[21:24] = one
        pa[21], pa[22], pa[23] = ry1, ry2, ry3
        in_maps.append({"ga": ga, "pa": pa})
    return in_maps


def kernel(preds, gts):
    from concourse.bass_utils import run_bass_kernel_spmd

    if "nc" not in _CACHE:
        _CACHE["nc"] = _build()
    nc = _CACHE["nc"]
    in_maps = _prep(preds, gts)
    res = run_bass_kernel_spmd(nc, in_maps, core_ids=list(range(B)))
    out = np.array(
        [res.results[b]["loss"][0, 0] for b in range(B)], dtype=np.float32
    )
    return out
